# revision 26
# baseline (speedup 1.0000x reference)
"""RoI mean-pooling (CustomRoIPooling) Trainium2 kernel.

Strategy
--------
out[b, n, c] = mask[b,n] * mean_{(h,w) in box_n} feature_map[b, c, h, w]

The box mean is a bilinear form: out[n, c] = s_n * sum_{hw} M[hw, n] * F[hw, c]
with M the 0/1 rectangle indicator of box n and s_n = mask_n / area_n.
On device this is a single PE (TensorEngine) matmul accumulation per image:
the flat spatial dim (H*W = 15200, padded to 119*128) is the contraction dim,
streamed through the PE in 128-row chunks; masks are the stationary operand
(128 x 100), features the moving operand (128 x 256), accumulating a
[100 x 256] fp32 PSUM tile. The per-box scale (mask/area) is applied by the
Scalar engine as a per-partition scale during the PSUM -> SBUF copy, so masks
stay exactly 0/1 and fp16 rounding only touches feature values
(resid_var vs the f32 reference ~5e-8).

The host pre-transposes the feature map to [hw, c] layout, builds the masks,
and pre-swizzles both into partition-major [128, chunk, free] layout so every
DMA is long contiguous runs at line rate. Data-parallel over batch:
16 images / 8 cores = 2 images per core.

Box coordinates are computed with the same jnp ops as the reference so the
float->int32 cast semantics (round-to-nearest on the neuron backend,
truncation on cpu) match the graded reference bit-exactly.
"""

import numpy as np

_B, _N, _C, _H, _W = 16, 100, 256, 100, 152
_HW = _H * _W               # 15200
_P = 128
_NCHUNK = (_HW + _P - 1) // _P   # 119
_HWP = _NCHUNK * _P         # 15232
_NCORES = 8
_BPC = _B // _NCORES        # batches per core = 2
_G = 24                     # chunks per feature DMA group (119 = 4*24 + 23)

_CACHE = {}


def _build_program(repeats=1, G=None, bufs=5, loop=1, dma="sync"):
    """Build the Bass/Tile program (same SPMD program for all 8 cores)."""
    import concourse.mybir as mybir
    from concourse import bacc
    from concourse.tile import TileContext

    if G is None:
        G = _G
    group_bounds = [(g0, min(g0 + G, _NCHUNK)) for g0 in range(0, _NCHUNK, G)]

    # Bacc (not raw Bass): its compile() runs generate_event_semaphores,
    # which splits multi-wait DMAs into event-semaphore pairs — TRN2 DMA
    # queue instructions accept at most one sync wait.
    nc = bacc.Bacc(None, target_bir_lowering=False)
    dma_eng = getattr(nc, dma)
    ft = nc.dram_tensor("ft", [_BPC, _P, _NCHUNK, _C], mybir.dt.float16,
                        kind="ExternalInput")
    mk = nc.dram_tensor("mk", [_BPC, _P, _NCHUNK, _N], mybir.dt.float8e4,
                        kind="ExternalInput")
    sc = nc.dram_tensor("sc", [_N, _BPC], mybir.dt.float32,
                        kind="ExternalInput")
    out = nc.dram_tensor("out", [_BPC, _N, _C], mybir.dt.float32,
                         kind="ExternalOutput")

    with TileContext(nc) as tc:
        with (
            tc.tile_pool(name="fpool", bufs=bufs) as fpool,
            tc.tile_pool(name="mpool", bufs=2) as mpool,
            tc.tile_pool(name="spool", bufs=1) as spool,
            tc.tile_pool(name="opool", bufs=2) as opool,
            tc.tile_pool(name="ppool", bufs=2, space="PSUM") as ppool,
        ):
            sct = spool.tile([_N, _BPC], mybir.dt.float32)
            dma_eng.dma_start(sct[:], sc[:])

            def body():
                for b in range(_BPC):
                    psum = ppool.tile([_N, _C], mybir.dt.float32)
                    # whole batch's masks in one 1.5 MB DMA (fp8: 11.9 KB/partition)
                    mtile = mpool.tile([_P, _NCHUNK, _N], mybir.dt.float8e4,
                                       tag="mtile")
                    dma_eng.dma_start(mtile[:], mk[b])
                    for (g0, g1) in group_bounds:
                        gw = g1 - g0
                        ftile = fpool.tile([_P, G, _C], mybir.dt.float16,
                                           tag="ftile")
                        dma_eng.dma_start(ftile[:, :gw, :], ft[b, :, g0:g1, :])
                        for j in range(gw):
                            k = g0 + j
                            nc.tensor.matmul(
                                psum[:],
                                mtile[:, k, :],
                                ftile[:, j, :],
                                start=(k == 0),
                                stop=(k == _NCHUNK - 1),
                            )
                    otile = opool.tile([_N, _C], mybir.dt.float32)
                    nc.scalar.mul(otile[:], psum[:], sct[:, b:b + 1])
                    dma_eng.dma_start(out[b], otile[:])

            if loop > 1:
                with tc.For_i(0, loop, 1,
                              hint_engines=(mybir.EngineType.PE,)):
                    body()
            else:
                for _ in range(repeats):
                    body()
    nc.compile()
    return nc


def _get_program(repeats=1, G=None, bufs=5, loop=1, dma="sync"):
    key = ("prog", repeats, G, bufs, loop, dma)
    if key not in _CACHE:
        _CACHE[key] = _build_program(repeats, G=G, bufs=bufs, loop=loop, dma=dma)
    return _CACHE[key]


def _box_coords(keypoints, original_H, original_W):
    """Replicate the reference's box-coordinate math with the same jnp ops
    (so float->int cast semantics match the graded reference exactly)."""
    import jax.numpy as jnp
    kp = jnp.asarray(np.asarray(keypoints, dtype=np.float32))
    sx = _W / int(original_W)
    sy = _H / int(original_H)
    x, y, w, h = kp[..., 0], kp[..., 1], kp[..., 2], kp[..., 3]
    xr = jnp.clip((x * sx).astype(jnp.int32), 0, _W - 1)
    yr = jnp.clip((y * sy).astype(jnp.int32), 0, _H - 1)
    wr = jnp.minimum(jnp.maximum((w * sx).astype(jnp.int32), 1), _W - xr)
    hr = jnp.minimum(jnp.maximum((h * sy).astype(jnp.int32), 1), _H - yr)
    return (np.asarray(xr), np.asarray(yr), np.asarray(wr), np.asarray(hr))


def _swizzle(arr_rows, free):
    """[B, HWP, free] -> partition-major [B, 128, NCHUNK, free] (contiguous)."""
    return np.ascontiguousarray(
        arr_rows.reshape(_B, _NCHUNK, _P, free).transpose(0, 2, 1, 3))


def _prep_inputs(feature_map, keypoints, mask, original_H, original_W):
    fm = np.asarray(feature_map, dtype=np.float32)
    valid = np.asarray(mask)
    xr, yr, wr, hr = _box_coords(keypoints, original_H, original_W)

    # features: [B, C, H, W] -> [B, HW, C] fp16, padded to HWP rows
    ft_rows = np.zeros((_B, _HWP, _C), np.float16)
    ft_rows[:, :_HW] = fm.transpose(0, 2, 3, 1).reshape(_B, _HW, _C)
    ft_sw = _swizzle(ft_rows, _C)

    # rectangle masks: [B, HW, N] in {0, 1}, fp8e4m3 (exact; mixed fp8
    # lhsT x fp16 rhs matmul verified bit-exact on HW)
    import ml_dtypes
    hh = np.arange(_H, dtype=np.int32)
    ww = np.arange(_W, dtype=np.int32)
    u = (hh[None, None, :] >= yr[:, :, None]) & (hh[None, None, :] < (yr + hr)[:, :, None])
    v = (ww[None, None, :] >= xr[:, :, None]) & (ww[None, None, :] < (xr + wr)[:, :, None])
    m_rows = np.zeros((_B, _HWP, _N), ml_dtypes.float8_e4m3fn)
    m_rows[:, :_HW] = (
        u.transpose(0, 2, 1)[:, :, None, :] & v.transpose(0, 2, 1)[:, None, :, :]
    ).reshape(_B, _HW, _N).astype(ml_dtypes.float8_e4m3fn)
    mk_sw = _swizzle(m_rows, _N)

    # per-box scale: mask / area
    area = (hr * wr).astype(np.float32)
    s = np.where(valid > 0, np.float32(1.0) / area, np.float32(0.0)).astype(np.float32)
    return ft_sw, mk_sw, s


def _make_in_maps(ft_sw, mk_sw, s):
    in_maps = []
    for i in range(_NCORES):
        b0 = i * _BPC
        in_maps.append({
            "ft": ft_sw[b0:b0 + _BPC],
            "mk": mk_sw[b0:b0 + _BPC],
            "sc": np.ascontiguousarray(s[b0:b0 + _BPC].T),
        })
    return in_maps


def _run(in_maps, repeats=1, G=None, bufs=5, dma="sync"):
    from concourse.bass_utils import run_bass_kernel_spmd
    nc = _get_program(repeats, G=G, bufs=bufs, dma=dma)
    res = run_bass_kernel_spmd(nc, in_maps, list(range(_NCORES)))
    return np.concatenate([r["out"] for r in res.results], axis=0)


def kernel(feature_map, keypoints, mask, original_H, original_W):
    ft_sw, mk_sw, s = _prep_inputs(feature_map, keypoints, mask,
                                   original_H, original_W)
    out = _run(_make_in_maps(ft_sw, mk_sw, s))
    return out.astype(np.float32, copy=False)
